# revision 1
# baseline (speedup 1.0000x reference)
"""Trainium2 Bass kernel: 5-point Laplacian smoothness loss over an 8192x8192
float32 matrix, sharded row-wise across 8 NeuronCores.

loss = 0.1 * sum_{i,j interior} | a[i,j] - 0.25*(a[i-1,j]+a[i+1,j]+a[i,j-1]+a[i,j+1]) |
     = 0.025 * sum | 4a[i,j] - (a[i-1,j]+a[i+1,j]+a[i,j-1]+a[i,j+1]) |

Per-core plan (memory-bound):
  - host casts the input to bf16 (loss error ~1e-6 in the 67M-term abs-sum;
    halves HBM traffic and runs the TensorE at full rate).
  - shard: 1026 rows (1024 center rows + 1 halo row each side; global edges
    zero-padded, their contributions dropped on the host).
  - tiles of 128 rows x 8192 cols with 2-row overlap (rows in partitions).
  - vertical stencil up+down-4c via TensorE matmul with a tridiagonal
    (1,-4,1) bf16 stationary matrix into PSUM (f32 accumulate).
  - horizontal l+r via VectorE shifted bf16 add (4x perf mode).
  - identity matmul accumulates the horizontal sum into the same PSUM bank.
  - abs+row-sum per 2048-col PSUM group: ScalarE activation(Abs, accum_out)
    for 3 groups, VectorE tensor_reduce(abs) for 1 group (engine balance).
  - output [1024, 4] per-row group sums per core; host masks edge rows and
    does the final float64 sum * 0.025.
"""

import numpy as np

N = 8192
NCORES = 8
ROWS_PER_CORE = N // NCORES          # 1024 center rows per core
SHARD_ROWS = ROWS_PER_CORE + 2       # 1026 with halo
NGROUPS = 4
LAMBDA_SMOOTH = 0.1

# center-column chunks (512-wide, last 510) grouped by 4 into PSUM groups
_CHUNKS = [(1 + 512 * i, min(1 + 512 * (i + 1), 8191)) for i in range(16)]
_GROUPS = [(_CHUNKS[4 * g][0], _CHUNKS[4 * g + 3][1], _CHUNKS[4 * g:4 * g + 4])
           for g in range(NGROUPS)]
DVE_GROUP = 3    # this group abs-reduced on VectorE, others on ScalarE

# tiles: (shard_row0, nrows, out_row0, n_center), 128-row tiles with 2-row
# overlap (126 new center rows each). First tile is small so the first DMA
# lands quickly and compute ramps up; a partial tail tile covers the rest.
def _make_tiles(first=32):
    tiles = [(0, first, 0, first - 2)]
    out = first - 2
    while out < ROWS_PER_CORE:
        ncen = min(126, ROWS_PER_CORE - out)
        tiles.append((out, ncen + 2, out, ncen))
        out += ncen
    return tiles


_TILES = _make_tiles(first=128)

_NC_CACHE = {}


def _build_nc(reps=1, mode="full"):
    """mode: 'full' | 'dma' (loads+store only) | 'dve' (+hsum) |
    'pe' (+matmuls, no abs-reduce) | 'noact' (DVE reduce only)."""
    import concourse.tile as tile
    from concourse import bacc, mybir

    f32 = mybir.dt.float32
    bf16 = mybir.dt.bfloat16
    nc = bacc.Bacc("TRN2", target_bir_lowering=False, debug=False)
    a_dram = nc.declare_dram_parameter("a", [SHARD_ROWS, N], bf16, isOutput=False)
    w_dram = nc.declare_dram_parameter("w", [256, 128], bf16, isOutput=False)
    out_dram = nc.declare_dram_parameter("out", [ROWS_PER_CORE, NGROUPS], f32,
                                         isOutput=True)

    with tile.TileContext(nc) as tc:
        with (
            tc.tile_pool(name="wpool", bufs=1) as wpool,
            tc.tile_pool(name="apool", bufs=6) as apool,
            tc.tile_pool(name="hpool", bufs=4) as hpool,
            tc.tile_pool(name="scpool", bufs=3) as scpool,
            tc.tile_pool(name="rpool", bufs=3) as rpool,
            tc.tile_pool(name="pspool", bufs=2, space="PSUM") as pspool,
        ):
            # stationary matrices: cols 0:128 tridiag(1,-4,1), cols 128:256 identity
            w_t = wpool.tile([128, 256], bf16)
            nc.gpsimd.dma_start(out=w_t[:, 0:128], in_=w_dram[0:128, :])
            nc.gpsimd.dma_start(out=w_t[:, 128:256], in_=w_dram[128:256, :])

            for _rep in range(reps):
                for ti, (r0, nrows, out0, ncen) in enumerate(_TILES):
                    a_t = apool.tile([128, N], bf16, tag="a")
                    nc.sync.dma_start(out=a_t[:nrows, :],
                                      in_=a_dram[r0:r0 + nrows, :])

                    if mode in ("full", "noact"):
                        racc = rpool.tile([128, NGROUPS], f32, tag="racc")
                    else:
                        racc = None
                    s4 = w_t[:nrows, 0:nrows]
                    ident = w_t[:nrows, 128:128 + nrows]

                    for gi, (gs, ge, chunks) in enumerate(_GROUPS):
                        gw = ge - gs
                        if mode != "dma":
                            h_t = hpool.tile([128, 2048], bf16, tag="h")
                            nc.vector.tensor_add(h_t[:nrows, :gw],
                                                 a_t[:nrows, gs - 1:gs - 1 + gw],
                                                 a_t[:nrows, gs + 1:gs + 1 + gw])
                        if mode in ("dma", "dve"):
                            continue
                        ps = pspool.tile([128, 2048], f32, tag="ps")
                        for (cs, ce) in chunks:
                            cw = ce - cs
                            o = cs - gs
                            nc.tensor.matmul(ps[:nrows, o:o + cw], s4,
                                             a_t[:nrows, cs:ce],
                                             start=True, stop=False)
                        for (cs, ce) in chunks:
                            cw = ce - cs
                            o = cs - gs
                            nc.tensor.matmul(ps[:nrows, o:o + cw], ident,
                                             h_t[:nrows, o:o + cw],
                                             start=False, stop=True)
                        if mode == "pe":
                            continue
                        if gi != DVE_GROUP and mode != "noact":
                            sc = scpool.tile([128, 2048], f32, tag="sc")
                            nc.scalar.activation(
                                sc[:nrows, :gw], ps[:nrows, :gw],
                                mybir.ActivationFunctionType.Abs,
                                accum_out=racc[:nrows, gi:gi + 1])
                        else:
                            nc.vector.tensor_reduce(
                                racc[:nrows, gi:gi + 1], ps[:nrows, :gw],
                                axis=mybir.AxisListType.X,
                                op=mybir.AluOpType.add,
                                apply_absolute_value=True)

                    if racc is not None:
                        nc.gpsimd.dma_start(out=out_dram[out0:out0 + ncen, :],
                                            in_=racc[1:1 + ncen, :])
    nc.compile()
    return nc


def _get_nc(reps=1, mode="full"):
    key = (reps, mode)
    if key not in _NC_CACHE:
        _NC_CACHE[key] = _build_nc(reps, mode)
    return _NC_CACHE[key]


def _weight_matrix():
    import ml_dtypes
    w = np.zeros((256, 128), dtype=np.float32)
    idx = np.arange(128)
    w[idx, idx] = -4.0
    w[idx[:-1], idx[:-1] + 1] = 1.0
    w[idx[1:], idx[1:] - 1] = 1.0
    w[128 + idx, idx] = 1.0
    return w.astype(ml_dtypes.bfloat16)


def _make_shards(adj):
    """Per-core [1026, 8192] bf16 shards with 1-row halo; zero rows at the
    global top/bottom edges (their center rows are masked out on the host)."""
    import ml_dtypes
    adj_bf = adj.astype(ml_dtypes.bfloat16)
    shards = []
    zrow = np.zeros((1, N), dtype=ml_dtypes.bfloat16)
    for k in range(NCORES):
        lo = k * ROWS_PER_CORE - 1
        hi = (k + 1) * ROWS_PER_CORE + 1
        parts = []
        if lo < 0:
            parts.append(zrow)
        parts.append(adj_bf[max(lo, 0):hi])
        if hi > N:
            parts.append(zrow)
        shard = np.ascontiguousarray(np.concatenate(parts, axis=0))
        assert shard.shape == (SHARD_ROWS, N)
        shards.append(shard)
    return shards


def _host_reduce(results):
    total = 0.0
    for k in range(NCORES):
        out = np.asarray(results[k]["out"], dtype=np.float64)
        # out row r <-> global center row k*1024 + r; valid iff 1 <= g <= 8190
        if k == 0:
            out = out[1:]
        if k == NCORES - 1:
            out = out[:-1]
        total += out.sum()
    return np.asarray(LAMBDA_SMOOTH * 0.25 * total, dtype=np.float32)


def kernel(adj: np.ndarray) -> np.ndarray:
    import time
    from concourse.bass_utils import run_bass_kernel_spmd

    adj = np.asarray(adj, dtype=np.float32)
    assert adj.shape == (N, N)

    nc = _get_nc()
    w = _weight_matrix()
    in_maps = [{"a": shard, "w": w} for shard in _make_shards(adj)]
    last_err = None
    for attempt in range(3):
        try:
            res = run_bass_kernel_spmd(nc, in_maps, list(range(NCORES)))
            return _host_reduce(res.results)
        except Exception as e:  # transient accelerator failures: back off, retry
            last_err = e
            time.sleep(45 * (attempt + 1))
    raise last_err



# revision 3
# speedup vs baseline: 1.0540x; 1.0540x over previous
"""Trainium2 Bass kernel: 5-point Laplacian smoothness loss over an 8192x8192
float32 matrix, sharded row-wise across 8 NeuronCores.

loss = 0.1 * sum_{i,j interior} | a[i,j] - 0.25*(a[i-1,j]+a[i+1,j]+a[i,j-1]+a[i,j+1]) |
     = 0.025 * sum | 4a[i,j] - (a[i-1,j]+a[i+1,j]+a[i,j-1]+a[i,j+1]) |

Per-core plan (measured-engine-balanced):
  - host casts the input to bf16 (loss error ~1e-6 in the 67M-term abs-sum).
  - shard: 1026 rows (1024 center + 1 halo row each side; global edges
    zero-padded, contributions dropped on the host).
  - tiles of 128 rows x 8192 cols with 2-row overlap (rows in partitions).
  - vertical stencil up-4c+down via TensorE matmul with tridiagonal
    (1,-4,1) bf16 stationary into PSUM (f32 accumulate).
  - horizontal l+r: for FG_GROUPS the PE adds it directly with two
    identity matmuls on +-1-column-shifted views of the input (PE has
    measured slack; DVE is the bottleneck); for the remaining groups a
    DVE shifted bf16 add produces h, accumulated via one identity matmul.
  - abs+row-sum from PSUM split between ScalarE activation(Abs, accum_out)
    and VectorE tensor_reduce(abs) per REDUCE_PLAN (measured rates:
    ScalarE ~2.0us, DVE ~2.26us per [128,2048] group).
  - output [1024, NR] per-row partial sums per core; host masks edge rows
    and does the final float64 sum * 0.025.
"""

import numpy as np

N = 8192
NCORES = 8
ROWS_PER_CORE = N // NCORES          # 1024 center rows per core
SHARD_ROWS = ROWS_PER_CORE + 2       # 1026 with halo
LAMBDA_SMOOTH = 0.1

# center-column chunks (512-wide, last 510) grouped by 4 into PSUM groups
_CHUNKS = [(1 + 512 * i, min(1 + 512 * (i + 1), 8191)) for i in range(16)]
_GROUPS = [(_CHUNKS[4 * g][0], _CHUNKS[4 * g + 3][1], _CHUNKS[4 * g:4 * g + 4])
           for g in range(4)]

# Groups whose horizontal add runs on the PE (double identity matmuls on
# shifted views); the rest use a DVE h-add + single identity matmul.
FG_GROUPS = (0, 1)
# abs-reduce plan: per group, list of (engine, col_lo, col_hi, racc_col).
# 'S' = ScalarE activation(Abs, accum), 'V' = VectorE tensor_reduce(abs).
REDUCE_PLAN = {
    0: [("S", 0, 2048, 0)],
    1: [("S", 0, 2048, 1)],
    2: [("V", 0, 896, 2), ("S", 896, 2048, 3)],
    3: [("V", 0, 2046, 4)],
}
NR = 5  # racc columns

# h-tile spans the DVE-h groups' center columns (must be contiguous groups)
_DVE_H_GROUPS = tuple(g for g in range(4) if g not in FG_GROUPS)
if _DVE_H_GROUPS:
    _HB = _GROUPS[_DVE_H_GROUPS[0]][0]          # first center col covered by h
    _HE = _GROUPS[_DVE_H_GROUPS[-1]][1]         # one past last
    _HW = _HE - _HB
else:
    _HB = _HE = _HW = 0

# tiles: (shard_row0, nrows, out_row0, n_center), 128-row tiles with 2-row
# overlap (126 new center rows each); partial tail tile covers the rest.
def _make_tiles(first=128):
    tiles = [(0, first, 0, first - 2)]
    out = first - 2
    while out < ROWS_PER_CORE:
        ncen = min(126, ROWS_PER_CORE - out)
        tiles.append((out, ncen + 2, out, ncen))
        out += ncen
    return tiles


_TILES = _make_tiles(first=128)

_NC_CACHE = {}


def _build_nc(reps=1, mode="full"):
    """mode: 'full' | 'dma' (loads only) | 'dve' (+h-adds) |
    'pe' (+matmuls, no abs-reduce)."""
    import concourse.tile as tile
    from concourse import bacc, mybir

    f32 = mybir.dt.float32
    bf16 = mybir.dt.bfloat16
    nc = bacc.Bacc("TRN2", target_bir_lowering=False, debug=False)
    a_dram = nc.declare_dram_parameter("a", [SHARD_ROWS, N], bf16, isOutput=False)
    w_dram = nc.declare_dram_parameter("w", [256, 128], bf16, isOutput=False)
    out_dram = nc.declare_dram_parameter("out", [ROWS_PER_CORE, NR], f32,
                                         isOutput=True)

    with tile.TileContext(nc) as tc:
        with (
            tc.tile_pool(name="wpool", bufs=1) as wpool,
            tc.tile_pool(name="apool", bufs=6) as apool,
            tc.tile_pool(name="hpool", bufs=3) as hpool,
            tc.tile_pool(name="scpool", bufs=3) as scpool,
            tc.tile_pool(name="rpool", bufs=3) as rpool,
            tc.tile_pool(name="pspool", bufs=2, space="PSUM") as pspool,
        ):
            # stationary matrices: cols 0:128 tridiag(1,-4,1), cols 128:256 identity
            w_t = wpool.tile([128, 256], bf16)
            nc.gpsimd.dma_start(out=w_t[:, 0:128], in_=w_dram[0:128, :])
            nc.gpsimd.dma_start(out=w_t[:, 128:256], in_=w_dram[128:256, :])

            for _rep in range(reps):
                for ti, (r0, nrows, out0, ncen) in enumerate(_TILES):
                    a_t = apool.tile([128, N], bf16, tag="a")
                    nc.sync.dma_start(out=a_t[:nrows, :],
                                      in_=a_dram[r0:r0 + nrows, :])
                    if mode == "dma":
                        continue

                    # fused DVE h-add over all DVE-h groups' columns
                    h_t = None
                    if _HW:
                        h_t = hpool.tile([128, _HW], bf16, tag="h")
                        nc.vector.tensor_add(
                            h_t[:nrows, :_HW],
                            a_t[:nrows, _HB - 1:_HB - 1 + _HW],
                            a_t[:nrows, _HB + 1:_HB + 1 + _HW])
                    if mode == "dve":
                        continue

                    if mode == "full":
                        racc = rpool.tile([128, NR], f32, tag="racc")
                    else:
                        racc = None
                    s4 = w_t[:nrows, 0:nrows]
                    ident = w_t[:nrows, 128:128 + nrows]

                    for gi, (gs, ge, chunks) in enumerate(_GROUPS):
                        gw = ge - gs
                        ps = pspool.tile([128, 2048], f32, tag="ps")
                        for (cs, ce) in chunks:
                            cw = ce - cs
                            o = cs - gs
                            nc.tensor.matmul(ps[:nrows, o:o + cw], s4,
                                             a_t[:nrows, cs:ce],
                                             start=True, stop=False)
                        if gi in FG_GROUPS:
                            for (cs, ce) in chunks:
                                cw = ce - cs
                                o = cs - gs
                                nc.tensor.matmul(ps[:nrows, o:o + cw], ident,
                                                 a_t[:nrows, cs - 1:ce - 1],
                                                 start=False, stop=False)
                            for (cs, ce) in chunks:
                                cw = ce - cs
                                o = cs - gs
                                nc.tensor.matmul(ps[:nrows, o:o + cw], ident,
                                                 a_t[:nrows, cs + 1:ce + 1],
                                                 start=False, stop=True)
                        else:
                            for (cs, ce) in chunks:
                                cw = ce - cs
                                o = cs - gs
                                nc.tensor.matmul(ps[:nrows, o:o + cw], ident,
                                                 h_t[:nrows, cs - _HB:ce - _HB],
                                                 start=False, stop=True)
                        if mode != "full":
                            continue
                        for (eng, lo, hi, rc) in REDUCE_PLAN[gi]:
                            if eng == "S":
                                sc = scpool.tile([128, 2048], f32, tag="sc")
                                nc.scalar.activation(
                                    sc[:nrows, :hi - lo], ps[:nrows, lo:hi],
                                    mybir.ActivationFunctionType.Abs,
                                    accum_out=racc[:nrows, rc:rc + 1])
                            else:
                                nc.vector.tensor_reduce(
                                    racc[:nrows, rc:rc + 1], ps[:nrows, lo:hi],
                                    axis=mybir.AxisListType.X,
                                    op=mybir.AluOpType.add,
                                    apply_absolute_value=True)

                    if racc is not None:
                        nc.gpsimd.dma_start(out=out_dram[out0:out0 + ncen, :],
                                            in_=racc[1:1 + ncen, :])
    nc.compile()
    return nc


def _get_nc(reps=1, mode="full"):
    key = (reps, mode)
    if key not in _NC_CACHE:
        _NC_CACHE[key] = _build_nc(reps, mode)
    return _NC_CACHE[key]


def _weight_matrix():
    import ml_dtypes
    w = np.zeros((256, 128), dtype=np.float32)
    idx = np.arange(128)
    w[idx, idx] = -4.0
    w[idx[:-1], idx[:-1] + 1] = 1.0
    w[idx[1:], idx[1:] - 1] = 1.0
    w[128 + idx, idx] = 1.0
    return w.astype(ml_dtypes.bfloat16)


def _make_shards(adj):
    """Per-core [1026, 8192] bf16 shards with 1-row halo; zero rows at the
    global top/bottom edges (their center rows are masked out on the host)."""
    import ml_dtypes
    adj_bf = adj.astype(ml_dtypes.bfloat16)
    shards = []
    zrow = np.zeros((1, N), dtype=ml_dtypes.bfloat16)
    for k in range(NCORES):
        lo = k * ROWS_PER_CORE - 1
        hi = (k + 1) * ROWS_PER_CORE + 1
        parts = []
        if lo < 0:
            parts.append(zrow)
        parts.append(adj_bf[max(lo, 0):hi])
        if hi > N:
            parts.append(zrow)
        shard = np.ascontiguousarray(np.concatenate(parts, axis=0))
        assert shard.shape == (SHARD_ROWS, N)
        shards.append(shard)
    return shards


def _host_reduce(results):
    total = 0.0
    for k in range(NCORES):
        out = np.asarray(results[k]["out"], dtype=np.float64)
        # out row r <-> global center row k*1024 + r; valid iff 1 <= g <= 8190
        if k == 0:
            out = out[1:]
        if k == NCORES - 1:
            out = out[:-1]
        total += out.sum()
    return np.asarray(LAMBDA_SMOOTH * 0.25 * total, dtype=np.float32)


def kernel(adj: np.ndarray) -> np.ndarray:
    import time
    from concourse.bass_utils import run_bass_kernel_spmd

    adj = np.asarray(adj, dtype=np.float32)
    assert adj.shape == (N, N)

    nc = _get_nc()
    w = _weight_matrix()
    in_maps = [{"a": shard, "w": w} for shard in _make_shards(adj)]
    last_err = None
    for attempt in range(3):
        try:
            res = run_bass_kernel_spmd(nc, in_maps, list(range(NCORES)))
            return _host_reduce(res.results)
        except Exception as e:  # transient accelerator failures: back off, retry
            last_err = e
            time.sleep(45 * (attempt + 1))
    raise last_err
